# revision 19
# baseline (speedup 1.0000x reference)
"""Graphormer encoder layer on 8 Trainium2 NeuronCores.

Sharding: rows (nodes) split 512-per-core (node parallel). Each core
computes full K/V for all heads from replicated hiddenT, Q for its own
512 rows, then biased attention in a transposed (m-partition) layout so
softmax's denominator falls out of a ones-column in the V matmul.

Speed structure (vs the fp32 version):
- every big matmul uses bf16 operands (1 cyc/row on PE instead of 4).
- spatial bias is applied multiplicatively: the host sends
  EB = exp(spa_emb[dist]) (bf16), so the kernel computes
  P = exp(S) * EB with one scalar_tensor_tensor in the DVE 2x perf mode
  (all operands 2-byte SBUF), and exp() reads scores straight out of
  PSUM on the ACT engine in 1024-wide tiles.
- layernorm runs on DVE except a [128,1] sqrt; activation-table loads
  are limited to exp once, sqrt runs, and one gelu block (alternating
  table-based functions costs ~1.3us per switch).
- K/Q head bands live at partition bases {0,32,64} (PE constraint),
  three tiles of up to 3 heads each, so projections run as wide
  [96,512] matmuls.
"""
import math
import numpy as np

import concourse.bass as bass
import concourse.bacc as bacc
import concourse.mybir as mybir
import concourse.tile as tile
from concourse import masks
from concourse.bass_utils import run_bass_kernel_spmd

N = 4096
C = 256
H = 8
Dh = 32
E = 65536
MAX_DIST = 4
MAX_DEG = 32
F = 512          # FF_MULT * C
EPS = 1e-5
NCORES = 8
R = N // NCORES  # 512 rows per core
MB = N // 128    # 32 m-blocks
QQ = MB // 4     # 8 quad-blocks (2048-wide softmax tiles)
NB = R // 128    # 4 n-blocks per core
VW = Dh + 1      # 33: V columns + ones column
VWP = Dh + 2     # 34: padded V slot width

f32 = mybir.dt.float32
bf16 = mybir.dt.bfloat16
AF = mybir.ActivationFunctionType
OP = mybir.AluOpType
AX = mybir.AxisListType.X

# head -> (tile, base partition); bases limited to {0, 32, 64}
TSLICE = [(0, 96), (96, 96), (192, 64)]  # weight col (start, width) per tile
HMAP = {h: (h // 3 if h < 6 else 2, (h % 3 if h < 6 else h - 6) * 32)
        for h in range(H)}


def _build_program():
    nc = bacc.Bacc("TRN2", target_bir_lowering=False, debug=False,
                   num_devices=NCORES)

    hT_d = nc.dram_tensor("hiddenT", [C, N], bf16, kind="ExternalInput")
    hTr_d = nc.dram_tensor("hTrows", [C, R], bf16, kind="ExternalInput")
    xr_d = nc.dram_tensor("xrows", [R, C], f32, kind="ExternalInput")
    # EB[h, qq, p, (pair, t, n)] = exp(spa[dist[m, n_glob], h]) with
    # m = qq*512 + pair*256 + t*128 + p  (host-prepared layout)
    eb_d = nc.dram_tensor("ebT", [H, QQ, 128, 2048], bf16,
                          kind="ExternalInput")
    wq_d = nc.dram_tensor("Wq", [C, C], bf16, kind="ExternalInput")  # /sqrt(D)
    wk_d = nc.dram_tensor("Wk", [C, C], bf16, kind="ExternalInput")
    wv_d = nc.dram_tensor("Wv", [C, C], bf16, kind="ExternalInput")
    wo_d = nc.dram_tensor("Wo", [C, C], bf16, kind="ExternalInput")
    wf1_d = nc.dram_tensor("Wf1", [C, F], bf16, kind="ExternalInput")
    wf2_d = nc.dram_tensor("Wf2", [F, C], bf16, kind="ExternalInput")
    # small vectors pre-replicated to 128 partitions on the host
    g1_d = nc.dram_tensor("g1r", [128, C], f32, kind="ExternalInput")
    b1_d = nc.dram_tensor("b1r", [128, C], f32, kind="ExternalInput")
    g2_d = nc.dram_tensor("g2r", [128, C], f32, kind="ExternalInput")
    b2_d = nc.dram_tensor("b2r", [128, C], f32, kind="ExternalInput")
    bo_d = nc.dram_tensor("bor", [128, C], f32, kind="ExternalInput")
    bf1_d = nc.dram_tensor("bf1r", [128, F], f32, kind="ExternalInput")
    bf2_d = nc.dram_tensor("bf2r", [128, C], f32, kind="ExternalInput")
    out_d = nc.dram_tensor("out", [R, C], f32, kind="ExternalOutput")

    with tile.TileContext(nc) as tc:
        with (
            tc.tile_pool(name="pers", bufs=1) as pers,
            tc.tile_pool(name="work", bufs=2) as work,
            tc.tile_pool(name="ps", bufs=1, space=bass.MemorySpace.PSUM) as ps,
        ):
            identb = pers.tile([128, 128], bf16, tag="identb", name="identb")
            masks.make_identity(nc, identb[:])

            # K^T/Q^T in 3 tiles of <=3 heads (bands at bases 0/32/64)
            kt = [pers.tile([w, N], bf16, tag=f"kt{t}", name=f"kt{t}")
                  for t, (_, w) in enumerate(TSLICE)]
            qt = [pers.tile([w, R], bf16, tag=f"qt{t}", name=f"qt{t}")
                  for t, (_, w) in enumerate(TSLICE)]
            # V (+ones col): [p, pair, t, h, d|1|pad] bf16
            vext = pers.tile([128, (MB // 2) * 2 * H * VWP], bf16, tag="vext",
                             name="vext")
            vext5 = vext.rearrange("p (q t h e) -> p q t h e", q=MB // 2, t=2,
                                   h=H)
            pacc = [pers.tile([128, C], f32, tag=f"pacc{i}", name=f"pacc{i}")
                    for i in range(NB)]
            wo = [pers.tile([32, C], bf16, tag=f"wo{i}", name=f"wo{i}")
                  for i in range(H)]
            wf1 = [pers.tile([128, F], bf16, tag=f"wf1_{i}", name=f"wf1_{i}")
                   for i in range(2)]
            wf2 = [pers.tile([128, C], bf16, tag=f"wf2_{i}", name=f"wf2_{i}")
                   for i in range(4)]
            reps = {}
            for nm, dram, w in (("g1", g1_d, C), ("b1", b1_d, C),
                                ("g2", g2_d, C), ("b2", b2_d, C),
                                ("bo", bo_d, C), ("bf1", bf1_d, F),
                                ("bf2", bf2_d, C)):
                t = pers.tile([128, w], f32, tag=f"rep_{nm}",
                              name=f"rep_{nm}")
                nc.sync.dma_start(t[:], dram[:, :])
                reps[nm] = t
            for i in range(H):
                nc.sync.dma_start(wo[i][:], wo_d[i * 32:(i + 1) * 32, :])
            for i in range(2):
                nc.sync.dma_start(wf1[i][:], wf1_d[i * 128:(i + 1) * 128, :])
            for i in range(4):
                nc.sync.dma_start(wf2[i][:], wf2_d[i * 128:(i + 1) * 128, :])

            # ---- Phase A: Q/K/V projections (bf16) ----
            htr = [pers.tile([128, R], bf16, tag=f"htr{i}", name=f"htr{i}")
                   for i in range(2)]
            wq = [pers.tile([128, C], bf16, tag=f"wq{i}", name=f"wq{i}")
                  for i in range(2)]
            wk = [pers.tile([128, C], bf16, tag=f"wk{i}", name=f"wk{i}")
                  for i in range(2)]
            wv = [pers.tile([128, C], bf16, tag=f"wv{i}", name=f"wv{i}")
                  for i in range(2)]
            for i in range(2):
                nc.sync.dma_start(htr[i][:], hTr_d[i * 128:(i + 1) * 128, :])
                nc.sync.dma_start(wq[i][:], wq_d[i * 128:(i + 1) * 128, :])
                nc.sync.dma_start(wk[i][:], wk_d[i * 128:(i + 1) * 128, :])
                nc.sync.dma_start(wv[i][:], wv_d[i * 128:(i + 1) * 128, :])

            # ones column (32) + pad (33) for every (pair, t, h) slot
            nc.vector.memset(vext5[:, :, :, :, Dh:VWP], 1.0)

            for t3, (cs, w) in enumerate(TSLICE):
                pq = ps.tile([96, R], f32, tag="st", name="pq", bufs=2)
                for cc in range(2):
                    nc.tensor.matmul(pq[0:w, :],
                                     lhsT=wq[cc][:, cs:cs + w],
                                     rhs=htr[cc][:],
                                     start=(cc == 0), stop=(cc == 1))
                nc.vector.tensor_copy(qt[t3][:], pq[0:w, :])

            for j in range(8):  # 512-wide chunks of the m axis
                htc = [work.tile([128, 512], bf16, tag=f"htc{i}",
                                 name=f"htc{i}", bufs=2) for i in range(2)]
                for i in range(2):
                    nc.sync.dma_start(
                        htc[i][:], hT_d[i * 128:(i + 1) * 128,
                                        j * 512:(j + 1) * 512])
                for t3, (cs, w) in enumerate(TSLICE):
                    pk = ps.tile([96, 512], f32, tag="st", name="pk", bufs=2)
                    for cc in range(2):
                        nc.tensor.matmul(pk[0:w, :],
                                         lhsT=wk[cc][:, cs:cs + w],
                                         rhs=htc[cc][:],
                                         start=(cc == 0), stop=(cc == 1))
                    nc.vector.tensor_copy(kt[t3][:, j * 512:(j + 1) * 512],
                                          pk[0:w, :])
                # V rows in natural [m, (h d)] layout -> vext slots (ACT copy)
                for mq in range(4):
                    mb = j * 4 + mq
                    pv = ps.tile([128, C], f32, tag="st", name="pv", bufs=2)
                    for cc in range(2):
                        nc.tensor.matmul(
                            pv[:],
                            lhsT=htc[cc][:, mq * 128:(mq + 1) * 128],
                            rhs=wv[cc][:],
                            start=(cc == 0), stop=(cc == 1))
                    dst = vext5[:, mb // 2, mb % 2, :, 0:Dh]
                    src = pv[:].rearrange("p (h d) -> p h d", d=Dh)
                    nc.scalar.copy(dst, src)

            # pacc = x_rows + bo
            for nb in range(NB):
                xb = work.tile([128, C], f32, tag="xb", name="xb")
                nc.sync.dma_start(xb[:], xr_d[nb * 128:(nb + 1) * 128, :])
                nc.vector.tensor_tensor(pacc[nb][:], xb[:], reps["bo"][:],
                                        op=OP.add)

            # ---- Phase B: attention ----
            # Wq is pre-scaled by 1/sqrt(Dh); P = exp(S) * EB; the ones
            # column in vext accumulates the softmax denominator.
            # Software-pipelined: each step s=(h,qq) emits scores+exp+mult
            # for s and the P@V matmuls for s-1, so the PE's in-order queue
            # never stalls behind a not-yet-ready P tile.
            attps = {}
            pexs = {}

            def emit_pv(h, qq):
                for k in range(4):
                    mb = 4 * qq + k
                    nc.tensor.matmul(
                        attps[h][:],
                        lhsT=vext5[:, mb // 2, mb % 2, h, 0:VW],
                        rhs=pexs[(h, qq)][:, k * 512:(k + 1) * 512],
                        start=(mb == 0), stop=(mb == MB - 1))

            def emit_tail(h):
                atts = work.tile([VW, R], bf16, tag="atts", name="atts")
                nc.vector.tensor_copy(atts[:], attps[h][:])
                for nb in range(NB):
                    # denominator -> per-partition reciprocal via transpose
                    rtp = ps.tile([128, VW], bf16, tag="tp", name="rtp")
                    nc.tensor.transpose(
                        rtp[:], atts[0:VW, nb * 128:(nb + 1) * 128],
                        identb[0:VW, 0:VW])
                    rec = work.tile([128, 1], f32, tag="rec", name="rec")
                    nc.vector.reciprocal(rec[:], rtp[:, Dh:Dh + 1])
                    pop = ps.tile([128, C], f32, tag="st", name="pop",
                                  bufs=2)
                    nc.tensor.matmul(pop[:],
                                     lhsT=atts[0:Dh, nb * 128:(nb + 1) * 128],
                                     rhs=wo[h][:],
                                     start=True, stop=True)
                    # pacc += pop * rec  (normalize + accumulate)
                    nc.vector.scalar_tensor_tensor(
                        out=pacc[nb][:], in0=pop[:], scalar=rec[:],
                        in1=pacc[nb][:], op0=OP.mult, op1=OP.add)

            for s in range(H * QQ):
                h, qq = divmod(s, QQ)
                t3, band = HMAP[h]
                if qq == 0:
                    attps[h] = ps.tile([VW, R], f32, tag="attp",
                                       name="attp", bufs=2)
                ebt = work.tile([128, 2048], bf16, tag="ebt", name="ebt",
                                bufs=3)
                nc.sync.dma_start(ebt[:], eb_d[h, qq, :, :])
                et = work.tile([128, 2048], bf16, tag="et", name="et",
                               bufs=2)
                for half in range(2):
                    stp = ps.tile([128, 1024], f32, tag="st", name="stp",
                                  bufs=2)
                    for t in range(2):
                        mb = 4 * qq + 2 * half + t
                        nc.tensor.matmul(
                            stp[:, t * 512:(t + 1) * 512],
                            lhsT=kt[t3][band:band + 32,
                                        mb * 128:(mb + 1) * 128],
                            rhs=qt[t3][band:band + 32, :],
                            start=True, stop=True)
                    nc.scalar.activation(
                        et[:, half * 1024:(half + 1) * 1024], stp[:],
                        AF.Exp)
                pex = work.tile([128, 2048], bf16, tag="pex", name="pex",
                                bufs=2)
                pexs[(h, qq)] = pex
                nc.vector.tensor_tensor(pex[:], et[:], ebt[:], op=OP.mult)
                if s > 0:
                    ph, pqq = divmod(s - 1, QQ)
                    emit_pv(ph, pqq)
                    if pqq == QQ - 1:
                        emit_tail(ph)
            emit_pv(H - 1, QQ - 1)
            emit_tail(H - 1)

            # ---- Phase C: LN1 + FF + LN2, batched per-op across blocks ----
            # LN math on DVE; ACT only does the [128,1] sqrt (+ gelu later),
            # so activation tables load at most 3 times after the exp phase.
            def layer_norm(dst, src, gr, br):
                mun = work.tile([128, 1], f32, tag="mun", name="mun", bufs=4)
                nc.vector.reduce_sum(mun[:], src[:], axis=AX, negate=True)
                nc.vector.tensor_scalar_mul(mun[:], mun[:], 1.0 / C)  # -mean
                xc = work.tile([128, C], f32, tag="xc", name="xc")
                nc.vector.tensor_scalar(xc[:], src[:], mun[:], None,
                                        op0=OP.add)
                sq = work.tile([128, C], f32, tag="sq", name="sq")
                nc.vector.tensor_tensor(sq[:], xc[:], xc[:], op=OP.mult)
                var = work.tile([128, 1], f32, tag="var", name="var")
                nc.vector.reduce_sum(var[:], sq[:], axis=AX)
                nc.vector.tensor_scalar(var[:], var[:], 1.0 / C, EPS,
                                        op0=OP.mult, op1=OP.add)
                std = work.tile([128, 1], f32, tag="std", name="std")
                nc.scalar.sqrt(std[:], var[:])
                rstd = work.tile([128, 1], f32, tag="rstd", name="rstd",
                                 bufs=4)
                nc.vector.reciprocal(rstd[:], std[:])
                nc.vector.tensor_scalar(dst[:], xc[:], rstd[:], None,
                                        op0=OP.mult)
                nc.vector.tensor_tensor(dst[:], dst[:], gr[:], op=OP.mult)
                nc.vector.tensor_tensor(dst[:], dst[:], br[:], op=OP.add)

            h1 = [work.tile([128, C], f32, tag=f"h1_{nb}", name=f"h1_{nb}")
                  for nb in range(NB)]
            h1b = [work.tile([128, C], bf16, tag=f"h1b_{nb}",
                             name=f"h1b_{nb}") for nb in range(NB)]
            for nb in range(NB):
                layer_norm(h1[nb], pacc[nb], reps["g1"], reps["b1"])
                nc.gpsimd.tensor_copy(h1b[nb][:], h1[nb][:])
            gl2 = [work.tile([128, F], bf16, tag=f"gl2_{nb}",
                             name=f"gl2_{nb}") for nb in range(NB)]
            for nb in range(NB):
                ff1 = ps.tile([128, F], f32, tag="st", name="ff1", bufs=2)
                for cc in range(2):
                    tp = ps.tile([128, 128], bf16, tag="tp", name="tp")
                    nc.tensor.transpose(
                        tp[:], h1b[nb][:, cc * 128:(cc + 1) * 128], identb[:])
                    h1t = work.tile([128, 128], bf16, tag="h1t", name="h1t",
                                    bufs=2)
                    nc.vector.tensor_copy(h1t[:], tp[:])
                    nc.tensor.matmul(ff1[:], lhsT=h1t[:], rhs=wf1[cc][:],
                                     start=(cc == 0), stop=(cc == 1))
                gl = work.tile([128, F], f32, tag="gl", name="gl")
                nc.vector.tensor_tensor(gl[:], ff1[:], reps["bf1"][:],
                                        op=OP.add)
                nc.scalar.activation(gl2[nb][:], gl[:], AF.Gelu)
            for nb in range(NB):
                ff2 = ps.tile([128, C], f32, tag="attp", name="ff2", bufs=2)
                for fc in range(4):
                    tp = ps.tile([128, 128], bf16, tag="tp", name="tp2")
                    nc.tensor.transpose(
                        tp[:], gl2[nb][:, fc * 128:(fc + 1) * 128], identb[:])
                    gt = work.tile([128, 128], bf16, tag="gt", name="gt",
                                   bufs=2)
                    nc.vector.tensor_copy(gt[:], tp[:])
                    nc.tensor.matmul(ff2[:], lhsT=gt[:], rhs=wf2[fc][:],
                                     start=(fc == 0), stop=(fc == 3))
                y = work.tile([128, C], f32, tag="y", name="y")
                nc.vector.tensor_tensor(y[:], ff2[:], h1[nb][:], op=OP.add)
                nc.vector.tensor_tensor(y[:], y[:], reps["bf2"][:], op=OP.add)
                o = work.tile([128, C], f32, tag="o", name="o")
                layer_norm(o, y, reps["g2"], reps["b2"])
                nc.sync.dma_start(out_d[nb * 128:(nb + 1) * 128, :], o[:])

    if not nc.is_finalized():
        nc.finalize()
    return nc


_NC_CACHE = None


def _get_program():
    global _NC_CACHE
    if _NC_CACHE is None:
        _NC_CACHE = _build_program()
    return _NC_CACHE


def _host_prep(x, edge_index, deg_emb):
    x = np.ascontiguousarray(np.asarray(x, np.float32))
    ei = np.asarray(edge_index)
    row = np.asarray(ei[0], np.int64)
    col = np.asarray(ei[1], np.int64)
    deg = np.bincount(row, minlength=N) + np.bincount(col, minlength=N)
    deg = np.minimum(deg, MAX_DEG + 1)
    hidden = x + np.asarray(deg_emb, np.float32)[deg]

    import scipy.sparse as sp
    import scipy.sparse.csgraph as csg
    data = np.ones(E, np.float32)
    adj = sp.csr_matrix((data, (row, col)), shape=(N, N))
    d = csg.shortest_path(adj, method="D", unweighted=True, directed=False)
    dist = np.where(np.isfinite(d), d, MAX_DIST + 1)
    dist = np.minimum(dist, MAX_DIST + 1).astype(np.int32)
    return hidden, dist


def _prepare_in_maps(inputs):
    import ml_dtypes
    x = np.asarray(inputs["x"], np.float32)
    spa = np.asarray(inputs["spa_emb"], np.float32)        # [MAX_DIST+2, H]
    hidden, dist = _host_prep(x, inputs["edge_index"], inputs["deg_emb"])
    hiddenT = np.ascontiguousarray(hidden.T)               # [C, N]
    espa = np.exp(spa)                                     # [MAX_DIST+2, H]

    bf = ml_dtypes.bfloat16
    cvt = lambda a: np.ascontiguousarray(np.asarray(a, np.float32).astype(bf))
    rep = lambda v, w: np.ascontiguousarray(
        np.broadcast_to(np.asarray(v, np.float32).reshape(1, w), (128, w)))
    shared = {
        "hiddenT": cvt(hiddenT),
        "Wq": cvt(np.asarray(inputs["Wq"], np.float32) / math.sqrt(Dh)),
        "Wk": cvt(inputs["Wk"]),
        "Wv": cvt(inputs["Wv"]),
        "Wo": cvt(inputs["Wo"]),
        "Wf1": cvt(inputs["Wf1"]),
        "Wf2": cvt(inputs["Wf2"]),
        "g1r": rep(inputs["g1"], C), "b1r": rep(inputs["b1"], C),
        "g2r": rep(inputs["g2"], C), "b2r": rep(inputs["b2"], C),
        "bor": rep(inputs["bo"], C), "bf1r": rep(inputs["bf1"], F),
        "bf2r": rep(inputs["bf2"], C),
    }
    in_maps = []
    for c in range(NCORES):
        rows = slice(c * R, (c + 1) * R)
        # EB[h, m, n_local] = exp(spa[dist[m, n_glob], h]); reorder m so an
        # SBUF tile [p, (pair, t, n)] matches score-tile layout:
        # m = qq*512 + pair*256 + t*128 + p.
        ebt = espa[dist[:, rows]]                          # [N, R, H]
        ebt = ebt.transpose(2, 0, 1)                       # [H, N, R]
        ebt = ebt.reshape(H, QQ, 2, 2, 128, R)             # [H, qq, pr, t, p, n]
        ebt = ebt.transpose(0, 1, 4, 2, 3, 5)              # [H, qq, p, pr, t, n]
        ebt = np.ascontiguousarray(
            ebt.reshape(H, QQ, 128, 2048).astype(bf))
        m = dict(shared)
        m["hTrows"] = cvt(hiddenT[:, rows])
        m["xrows"] = np.ascontiguousarray(x[rows, :])
        m["ebT"] = ebt
        in_maps.append(m)
    return in_maps


def kernel(**inputs) -> np.ndarray:
    in_maps = _prepare_in_maps(inputs)
    nc = _get_program()
    res = run_bass_kernel_spmd(nc, in_maps, list(range(NCORES)))
    out = np.concatenate([res.results[c]["out"] for c in range(NCORES)],
                         axis=0)
    return out.astype(np.float32)


if __name__ == "__main__":
    rng = np.random.default_rng(0)
    demo = {
        "x": rng.standard_normal((N, C), np.float32),
        "edge_index": rng.integers(0, N, (2, E)).astype(np.int64),
        "deg_emb": rng.standard_normal((MAX_DEG + 2, C), np.float32) * .02,
        "spa_emb": rng.standard_normal((MAX_DIST + 2, H), np.float32) * .02,
    }
    for nm, shp in (("Wq", (C, C)), ("Wk", (C, C)), ("Wv", (C, C)),
                    ("Wo", (C, C)), ("Wf1", (C, F)), ("Wf2", (F, C))):
        demo[nm] = rng.standard_normal(shp, np.float32) * .02
    for nm, w in (("bq", C), ("bk", C), ("bv", C), ("bo", C),
                  ("b1", C), ("b2", C), ("bf1", F), ("bf2", C)):
        demo[nm] = np.zeros(w, np.float32)
    demo["g1"] = np.ones(C, np.float32)
    demo["g2"] = np.ones(C, np.float32)
    print(kernel(**demo).shape)


# revision 20
# speedup vs baseline: 1.0649x; 1.0649x over previous
"""Graphormer encoder layer on 8 Trainium2 NeuronCores.

Sharding: rows (nodes) split 512-per-core (node parallel). Each core
computes full K/V for all heads from replicated hiddenT, Q for its own
512 rows, then biased attention in a transposed (m-partition) layout so
softmax's denominator falls out of a ones-column in the V matmul.

The PE spends most of the run firmware-throttled to 1.2 GHz (8 cores
under sustained load), so PE work is halved wherever fp8 DoubleRow
applies without hurting accuracy-critical paths:
- QK^T scores run fp8 DoubleRow with the head dim split (p, t)=(16, 2);
  K/Q land in that layout via an fp8 staging tile + partition-shifting
  SBUF->SBUF DMAs on the gpsimd queue.
- K/V projections run fp8 DoubleRow over the two 128-deep halves of C.
- P@V stays bf16: fp8 P would force the EB multiply off the DVE 2x
  path, which costs more than DoubleRow saves.
- spatial bias is multiplicative: host sends EB = exp(spa[dist]) bf16;
  P = exp(S) * EB is one 2x tensor_tensor; exp reads scores straight
  from PSUM.
- layernorm uses bn_stats/bn_aggr on DVE plus a [128,1] sqrt on ACT,
  keeping activation-table loads to exp + sqrt + gelu + sqrt.
"""
import math
import numpy as np

import concourse.bass as bass
import concourse.bacc as bacc
import concourse.mybir as mybir
import concourse.tile as tile
from concourse import masks
from concourse.bass_utils import run_bass_kernel_spmd

N = 4096
C = 256
H = 8
Dh = 32
E = 65536
MAX_DIST = 4
MAX_DEG = 32
F = 512          # FF_MULT * C
EPS = 1e-5
NCORES = 8
R = N // NCORES  # 512 rows per core
MB = N // 128    # 32 m-blocks
QQ = MB // 4     # 8 quad-blocks (2048-wide softmax tiles)
NB = R // 128    # 4 n-blocks per core
VW = Dh + 1      # 33: V columns + ones column
VWP = Dh + 2     # 34: padded V slot width

f32 = mybir.dt.float32
bf16 = mybir.dt.bfloat16
fp8 = mybir.dt.float8e4
AF = mybir.ActivationFunctionType
OP = mybir.AluOpType
AX = mybir.AxisListType.X
DR = mybir.MatmulPerfMode.DoubleRow

# head -> (tile, base partition); bases limited to {0, 32, 64}
HMAP = {h: (h // 3 if h < 6 else 2, (h % 3 if h < 6 else h - 6) * 32)
        for h in range(H)}
NHEADS3 = [3, 3, 2]


def _build_program():
    nc = bacc.Bacc("TRN2", target_bir_lowering=False, debug=False,
                   num_devices=NCORES)

    hT8_d = nc.dram_tensor("hiddenT8", [C, N], fp8, kind="ExternalInput")
    hTr_d = nc.dram_tensor("hTrows", [C, R], bf16, kind="ExternalInput")
    xr_d = nc.dram_tensor("xrows", [R, C], f32, kind="ExternalInput")
    # EB[h, qq, p, (pair, t, n)] = exp(spa[dist[m, n_glob], h]) with
    # m = qq*512 + pair*256 + t*128 + p  (host-prepared layout)
    eb_d = nc.dram_tensor("ebT", [H, QQ, 128, 2048], bf16,
                          kind="ExternalInput")
    wq_d = nc.dram_tensor("Wq", [C, C], bf16, kind="ExternalInput")  # /sqrt(D)
    # Wk/Wv in fp8, reshaped so the two 128-deep C halves are adjacent
    # k-tiles: [128, 2, C]
    wk_d = nc.dram_tensor("Wk8", [128, 2 * C], fp8, kind="ExternalInput")
    wv_d = nc.dram_tensor("Wv8", [128, 2 * C], fp8, kind="ExternalInput")
    wo_d = nc.dram_tensor("Wo", [C, C], bf16, kind="ExternalInput")
    wf1_d = nc.dram_tensor("Wf1", [C, F], bf16, kind="ExternalInput")
    wf2_d = nc.dram_tensor("Wf2", [F, C], bf16, kind="ExternalInput")
    # small vectors pre-replicated to 128 partitions on the host
    g1_d = nc.dram_tensor("g1r", [128, C], f32, kind="ExternalInput")
    b1_d = nc.dram_tensor("b1r", [128, C], f32, kind="ExternalInput")
    g2_d = nc.dram_tensor("g2r", [128, C], f32, kind="ExternalInput")
    b2_d = nc.dram_tensor("b2r", [128, C], f32, kind="ExternalInput")
    bo_d = nc.dram_tensor("bor", [128, C], f32, kind="ExternalInput")
    bf1_d = nc.dram_tensor("bf1r", [128, F], f32, kind="ExternalInput")
    bf2_d = nc.dram_tensor("bf2r", [128, C], f32, kind="ExternalInput")
    out_d = nc.dram_tensor("out", [R, C], f32, kind="ExternalOutput")

    with tile.TileContext(nc) as tc:
        with (
            tc.tile_pool(name="pers", bufs=1) as pers,
            tc.tile_pool(name="work", bufs=2) as work,
            tc.tile_pool(name="ps", bufs=1, space=bass.MemorySpace.PSUM) as ps,
        ):
            identb = pers.tile([128, 128], bf16, tag="identb", name="identb")
            masks.make_identity(nc, identb[:])

            # K/Q in DoubleRow d-split layout: head h lives at partitions
            # band..band+16 of tile t3; d = t*16 + p.
            ktdr = [pers.tile([80, MB * 2 * 128], fp8, tag=f"ktdr{t}",
                              name=f"ktdr{t}") for t in range(3)]
            ktdr4 = [k.rearrange("p (mb t m) -> p mb t m", t=2, m=128)
                     for k in ktdr]
            qtdr = [pers.tile([80, 2 * R], fp8, tag=f"qtdr{t}",
                              name=f"qtdr{t}") for t in range(3)]
            qtdr3 = [q.rearrange("p (t n) -> p t n", t=2) for q in qtdr]
            # fp8 staging in natural [d, m] layout (4 heads per tile)
            kstag = [pers.tile([128, N], fp8, tag=f"kstag{g}",
                               name=f"kstag{g}") for g in range(2)]
            qstag = [pers.tile([128, R], fp8, tag=f"qstag{g}",
                               name=f"qstag{g}") for g in range(2)]
            # V (+ones col): [p, pair, t, h, d|1|pad] bf16
            vext = pers.tile([128, (MB // 2) * 2 * H * VWP], bf16, tag="vext",
                             name="vext")
            vext5 = vext.rearrange("p (q t h e) -> p q t h e", q=MB // 2, t=2,
                                   h=H)
            pacc = [pers.tile([128, C], f32, tag=f"pacc{i}", name=f"pacc{i}")
                    for i in range(NB)]
            wo = [pers.tile([32, C], bf16, tag=f"wo{i}", name=f"wo{i}")
                  for i in range(H)]
            wf1 = [pers.tile([128, F], bf16, tag=f"wf1_{i}", name=f"wf1_{i}")
                   for i in range(2)]
            wf2 = [pers.tile([128, C], bf16, tag=f"wf2_{i}", name=f"wf2_{i}")
                   for i in range(4)]
            reps = {}
            for nm, dram, w in (("g1", g1_d, C), ("b1", b1_d, C),
                                ("g2", g2_d, C), ("b2", b2_d, C),
                                ("bo", bo_d, C), ("bf1", bf1_d, F),
                                ("bf2", bf2_d, C)):
                t = pers.tile([128, w], f32, tag=f"rep_{nm}",
                              name=f"rep_{nm}")
                nc.sync.dma_start(t[:], dram[:, :])
                reps[nm] = t
            for i in range(H):
                nc.sync.dma_start(wo[i][:], wo_d[i * 32:(i + 1) * 32, :])
            for i in range(2):
                nc.sync.dma_start(wf1[i][:], wf1_d[i * 128:(i + 1) * 128, :])
            for i in range(4):
                nc.sync.dma_start(wf2[i][:], wf2_d[i * 128:(i + 1) * 128, :])

            # ---- Phase A: Q/K/V projections ----
            htr = [pers.tile([128, R], bf16, tag=f"htr{i}", name=f"htr{i}")
                   for i in range(2)]
            wq = [pers.tile([128, C], bf16, tag=f"wq{i}", name=f"wq{i}")
                  for i in range(2)]
            wk8 = pers.tile([128, 2 * C], fp8, tag="wk8", name="wk8")
            wv8 = pers.tile([128, 2 * C], fp8, tag="wv8", name="wv8")
            wk83 = wk8.rearrange("p (t c) -> p t c", t=2)
            wv83 = wv8.rearrange("p (t c) -> p t c", t=2)
            nc.sync.dma_start(wk8[:], wk_d[:, :])
            nc.sync.dma_start(wv8[:], wv_d[:, :])
            for i in range(2):
                nc.sync.dma_start(htr[i][:], hTr_d[i * 128:(i + 1) * 128, :])
                nc.sync.dma_start(wq[i][:], wq_d[i * 128:(i + 1) * 128, :])

            # ones column (32) + pad (33) for every (pair, t, h) slot
            nc.vector.memset(vext5[:, :, :, :, Dh:VWP], 1.0)

            # Q^T: bf16 matmul -> fp8 staging -> DR-layout remap DMAs
            for g in range(2):
                pq = ps.tile([128, R], f32, tag="st", name="pq", bufs=2)
                for cc in range(2):
                    nc.tensor.matmul(pq[:],
                                     lhsT=wq[cc][:, g * 128:(g + 1) * 128],
                                     rhs=htr[cc][:],
                                     start=(cc == 0), stop=(cc == 1))
                nc.vector.tensor_copy(qstag[g][:], pq[:])
            for h in range(H):
                t3, band = HMAP[h]
                g, i = divmod(h, 4)
                for t in range(2):
                    nc.gpsimd.dma_start(
                        qtdr3[t3][band:band + 16, t, :],
                        qstag[g][32 * i + 16 * t:32 * i + 16 * t + 16, :])

            # K: fp8 DoubleRow proj (C contracted as 2 k-tiles of 128)
            for j in range(8):  # 512-wide chunks of the m axis
                htc8 = work.tile([128, 2 * 512], fp8, tag="htc8",
                                 name="htc8", bufs=2)
                htc83 = htc8.rearrange("p (t m) -> p t m", t=2)
                for cc in range(2):
                    nc.sync.dma_start(
                        htc83[:, cc, :], hT8_d[cc * 128:(cc + 1) * 128,
                                               j * 512:(j + 1) * 512])
                for g in range(2):
                    pk = ps.tile([128, 512], f32, tag="st", name="pk", bufs=2)
                    nc.tensor.matmul(
                        pk[:],
                        lhsT=wk83[:, :, g * 128:(g + 1) * 128],
                        rhs=htc83[:, :, :],
                        perf_mode=DR, start=True, stop=True)
                    nc.vector.tensor_copy(kstag[g][:, j * 512:(j + 1) * 512],
                                          pk[:])
                # V rows in natural [m, (h d)] layout -> vext slots (ACT copy)
                for mq in range(4):
                    mb = j * 4 + mq
                    pv = ps.tile([128, C], f32, tag="attp", name="pv", bufs=2)
                    nc.tensor.matmul(
                        pv[:],
                        lhsT=htc83[:, :, mq * 128:(mq + 1) * 128],
                        rhs=wv83[:, :, :],
                        perf_mode=DR, start=True, stop=True)
                    dst = vext5[:, mb // 2, mb % 2, :, 0:Dh]
                    src = pv[:].rearrange("p (h d) -> p h d", d=Dh)
                    nc.scalar.copy(dst, src)

            # K remap into DoubleRow layout (partition-shifting DMAs on the
            # gpsimd SWDGE queue so they don't delay the EB stream)
            for h in range(H):
                t3, band = HMAP[h]
                g, i = divmod(h, 4)
                for t in range(2):
                    nc.gpsimd.dma_start(
                        ktdr4[t3][band:band + 16, :, t, :],
                        kstag[g][32 * i + 16 * t:32 * i + 16 * t + 16, :])

            # pacc = x_rows + bo
            for nb in range(NB):
                xb = work.tile([128, C], f32, tag="xb", name="xb")
                nc.sync.dma_start(xb[:], xr_d[nb * 128:(nb + 1) * 128, :])
                nc.vector.tensor_tensor(pacc[nb][:], xb[:], reps["bo"][:],
                                        op=OP.add)

            # ---- Phase B: attention ----
            # Software-pipelined: each step s=(h,qq) emits scores+exp+mult
            # for s and the P@V matmuls for s-1, so the PE's in-order queue
            # never stalls behind a not-yet-ready P tile.
            attps = {}
            pexs = {}

            def emit_pv(h, qq):
                for k in range(4):
                    mb = 4 * qq + k
                    nc.tensor.matmul(
                        attps[h][:],
                        lhsT=vext5[:, mb // 2, mb % 2, h, 0:VW],
                        rhs=pexs[(h, qq)][:, k * 512:(k + 1) * 512],
                        start=(mb == 0), stop=(mb == MB - 1))

            def emit_tail(h):
                atts = work.tile([VW, R], bf16, tag="atts", name="atts")
                nc.vector.tensor_copy(atts[:], attps[h][:])
                for nb in range(NB):
                    # denominator -> per-partition reciprocal via transpose
                    rtp = ps.tile([128, VW], bf16, tag="tp", name="rtp")
                    nc.tensor.transpose(
                        rtp[:], atts[0:VW, nb * 128:(nb + 1) * 128],
                        identb[0:VW, 0:VW])
                    rec = work.tile([128, 1], f32, tag="rec", name="rec")
                    nc.vector.reciprocal(rec[:], rtp[:, Dh:Dh + 1])
                    pop = ps.tile([128, C], f32, tag="st", name="pop",
                                  bufs=2)
                    nc.tensor.matmul(pop[:],
                                     lhsT=atts[0:Dh, nb * 128:(nb + 1) * 128],
                                     rhs=wo[h][:],
                                     start=True, stop=True)
                    # pacc += pop * rec  (normalize + accumulate)
                    nc.vector.scalar_tensor_tensor(
                        out=pacc[nb][:], in0=pop[:], scalar=rec[:],
                        in1=pacc[nb][:], op0=OP.mult, op1=OP.add)

            for s in range(H * QQ):
                h, qq = divmod(s, QQ)
                t3, band = HMAP[h]
                if qq == 0:
                    attps[h] = ps.tile([VW, R], f32, tag="attp",
                                       name="attp", bufs=2)
                ebt = work.tile([128, 2048], bf16, tag="ebt", name="ebt",
                                bufs=3)
                nc.sync.dma_start(ebt[:], eb_d[h, qq, :, :])
                et = work.tile([128, 2048], bf16, tag="et", name="et",
                               bufs=3)
                for half in range(2):
                    stp = ps.tile([128, 1024], f32, tag="st", name="stp",
                                  bufs=2)
                    for sl in range(2):
                        mb = 4 * qq + 2 * half + sl
                        nc.tensor.matmul(
                            stp[:, sl * 512:(sl + 1) * 512],
                            lhsT=ktdr4[t3][band:band + 16, mb, :, :],
                            rhs=qtdr3[t3][band:band + 16, :, :],
                            perf_mode=DR, start=True, stop=True)
                    nc.scalar.activation(
                        et[:, half * 1024:(half + 1) * 1024], stp[:],
                        AF.Exp)
                pex = work.tile([128, 2048], bf16, tag="pex", name="pex",
                                bufs=3)
                pexs[(h, qq)] = pex
                nc.vector.tensor_tensor(pex[:], et[:], ebt[:], op=OP.mult)
                if s > 0:
                    ph, pqq = divmod(s - 1, QQ)
                    emit_pv(ph, pqq)
                    if pqq == QQ - 1:
                        emit_tail(ph)
            emit_pv(H - 1, QQ - 1)
            emit_tail(H - 1)

            # ---- Phase C: LN1 + FF + LN2, batched per-op across blocks ----
            # LN stats via bn_stats/bn_aggr on DVE; ACT does the [128,1]
            # sqrt plus gelu, and the transpose-evac copies.
            def layer_norm(dst, src, gr, br):
                st6 = work.tile([128, 6], f32, tag="st6", name="st6")
                nc.vector.bn_stats(st6[:], src[:])
                mv = work.tile([128, 2], f32, tag="mv", name="mv")
                nc.vector.bn_aggr(mv[:], st6[:])
                var = work.tile([128, 1], f32, tag="var", name="var")
                nc.vector.tensor_scalar(var[:], mv[:, 1:2], EPS, None,
                                        op0=OP.add)
                std = work.tile([128, 1], f32, tag="std", name="std")
                nc.scalar.sqrt(std[:], var[:])
                rstd = work.tile([128, 1], f32, tag="rstd", name="rstd",
                                 bufs=4)
                nc.vector.reciprocal(rstd[:], std[:])
                nc.vector.tensor_scalar(dst[:], src[:], mv[:, 0:1], rstd[:],
                                        op0=OP.subtract, op1=OP.mult)
                nc.vector.tensor_tensor(dst[:], dst[:], gr[:], op=OP.mult)
                nc.vector.tensor_tensor(dst[:], dst[:], br[:], op=OP.add)

            h1 = [work.tile([128, C], f32, tag=f"h1_{nb}", name=f"h1_{nb}")
                  for nb in range(NB)]
            h1b = [work.tile([128, C], bf16, tag=f"h1b_{nb}",
                             name=f"h1b_{nb}") for nb in range(NB)]
            for nb in range(NB):
                layer_norm(h1[nb], pacc[nb], reps["g1"], reps["b1"])
                nc.gpsimd.tensor_copy(h1b[nb][:], h1[nb][:])
            gl2 = [work.tile([128, F], bf16, tag=f"gl2_{nb}",
                             name=f"gl2_{nb}") for nb in range(NB)]
            for nb in range(NB):
                ff1 = ps.tile([128, F], f32, tag="st", name="ff1", bufs=2)
                for cc in range(2):
                    tp = ps.tile([128, 128], bf16, tag="tp", name="tp")
                    nc.tensor.transpose(
                        tp[:], h1b[nb][:, cc * 128:(cc + 1) * 128], identb[:])
                    h1t = work.tile([128, 128], bf16, tag="h1t", name="h1t",
                                    bufs=2)
                    nc.scalar.copy(h1t[:], tp[:])
                    nc.tensor.matmul(ff1[:], lhsT=h1t[:], rhs=wf1[cc][:],
                                     start=(cc == 0), stop=(cc == 1))
                gl = work.tile([128, F], f32, tag="gl", name="gl")
                nc.vector.tensor_tensor(gl[:], ff1[:], reps["bf1"][:],
                                        op=OP.add)
                nc.scalar.activation(gl2[nb][:], gl[:], AF.Gelu)
            for nb in range(NB):
                ff2 = ps.tile([128, C], f32, tag="attp", name="ff2", bufs=2)
                for fc in range(4):
                    tp = ps.tile([128, 128], bf16, tag="tp", name="tp2")
                    nc.tensor.transpose(
                        tp[:], gl2[nb][:, fc * 128:(fc + 1) * 128], identb[:])
                    gt = work.tile([128, 128], bf16, tag="gt", name="gt",
                                   bufs=2)
                    nc.scalar.copy(gt[:], tp[:])
                    nc.tensor.matmul(ff2[:], lhsT=gt[:], rhs=wf2[fc][:],
                                     start=(fc == 0), stop=(fc == 3))
                y = work.tile([128, C], f32, tag="y", name="y")
                nc.vector.tensor_tensor(y[:], ff2[:], h1[nb][:], op=OP.add)
                nc.vector.tensor_tensor(y[:], y[:], reps["bf2"][:], op=OP.add)
                o = work.tile([128, C], f32, tag="o", name="o")
                layer_norm(o, y, reps["g2"], reps["b2"])
                nc.sync.dma_start(out_d[nb * 128:(nb + 1) * 128, :], o[:])

    if not nc.is_finalized():
        nc.finalize()
    return nc


_NC_CACHE = None


def _get_program():
    global _NC_CACHE
    if _NC_CACHE is None:
        _NC_CACHE = _build_program()
    return _NC_CACHE


def _host_prep(x, edge_index, deg_emb):
    x = np.ascontiguousarray(np.asarray(x, np.float32))
    ei = np.asarray(edge_index)
    row = np.asarray(ei[0], np.int64)
    col = np.asarray(ei[1], np.int64)
    deg = np.bincount(row, minlength=N) + np.bincount(col, minlength=N)
    deg = np.minimum(deg, MAX_DEG + 1)
    hidden = x + np.asarray(deg_emb, np.float32)[deg]

    import scipy.sparse as sp
    import scipy.sparse.csgraph as csg
    data = np.ones(E, np.float32)
    adj = sp.csr_matrix((data, (row, col)), shape=(N, N))
    d = csg.shortest_path(adj, method="D", unweighted=True, directed=False)
    dist = np.where(np.isfinite(d), d, MAX_DIST + 1)
    dist = np.minimum(dist, MAX_DIST + 1).astype(np.int32)
    return hidden, dist


def _prepare_in_maps(inputs):
    import ml_dtypes
    x = np.asarray(inputs["x"], np.float32)
    spa = np.asarray(inputs["spa_emb"], np.float32)        # [MAX_DIST+2, H]
    hidden, dist = _host_prep(x, inputs["edge_index"], inputs["deg_emb"])
    hiddenT = np.ascontiguousarray(hidden.T)               # [C, N]
    espa = np.exp(spa)                                     # [MAX_DIST+2, H]

    bf = ml_dtypes.bfloat16
    f8 = ml_dtypes.float8_e4m3
    cvt = lambda a: np.ascontiguousarray(np.asarray(a, np.float32).astype(bf))
    rep = lambda v, w: np.ascontiguousarray(
        np.broadcast_to(np.asarray(v, np.float32).reshape(1, w), (128, w)))

    def to_ktile(wm):  # [C, C] -> [128, 2, C] fp8 (C halves as k-tiles)
        a = np.asarray(wm, np.float32).reshape(2, 128, C).transpose(1, 0, 2)
        return np.ascontiguousarray(a.reshape(128, 2 * C).astype(f8))

    shared = {
        "hiddenT8": np.ascontiguousarray(hiddenT.astype(f8)),
        "Wq": cvt(np.asarray(inputs["Wq"], np.float32) / math.sqrt(Dh)),
        "Wk8": to_ktile(inputs["Wk"]),
        "Wv8": to_ktile(inputs["Wv"]),
        "Wo": cvt(inputs["Wo"]),
        "Wf1": cvt(inputs["Wf1"]),
        "Wf2": cvt(inputs["Wf2"]),
        "g1r": rep(inputs["g1"], C), "b1r": rep(inputs["b1"], C),
        "g2r": rep(inputs["g2"], C), "b2r": rep(inputs["b2"], C),
        "bor": rep(inputs["bo"], C), "bf1r": rep(inputs["bf1"], F),
        "bf2r": rep(inputs["bf2"], C),
    }
    in_maps = []
    for c in range(NCORES):
        rows = slice(c * R, (c + 1) * R)
        # EB[h, m, n_local] = exp(spa[dist[m, n_glob], h]); reorder m so an
        # SBUF tile [p, (pair, t, n)] matches score-tile layout:
        # m = qq*512 + pair*256 + t*128 + p.
        ebt = espa[dist[:, rows]]                          # [N, R, H]
        ebt = ebt.transpose(2, 0, 1)                       # [H, N, R]
        ebt = ebt.reshape(H, QQ, 2, 2, 128, R)             # [H, qq, pr, t, p, n]
        ebt = ebt.transpose(0, 1, 4, 2, 3, 5)              # [H, qq, p, pr, t, n]
        ebt = np.ascontiguousarray(
            ebt.reshape(H, QQ, 128, 2048).astype(bf))
        m = dict(shared)
        m["hTrows"] = cvt(hiddenT[:, rows])
        m["xrows"] = np.ascontiguousarray(x[rows, :])
        m["ebT"] = ebt
        in_maps.append(m)
    return in_maps


def kernel(**inputs) -> np.ndarray:
    in_maps = _prepare_in_maps(inputs)
    nc = _get_program()
    res = run_bass_kernel_spmd(nc, in_maps, list(range(NCORES)))
    out = np.concatenate([res.results[c]["out"] for c in range(NCORES)],
                         axis=0)
    return out.astype(np.float32)


if __name__ == "__main__":
    rng = np.random.default_rng(0)
    demo = {
        "x": rng.standard_normal((N, C), np.float32),
        "edge_index": rng.integers(0, N, (2, E)).astype(np.int64),
        "deg_emb": rng.standard_normal((MAX_DEG + 2, C), np.float32) * .02,
        "spa_emb": rng.standard_normal((MAX_DIST + 2, H), np.float32) * .02,
    }
    for nm, shp in (("Wq", (C, C)), ("Wk", (C, C)), ("Wv", (C, C)),
                    ("Wo", (C, C)), ("Wf1", (C, F)), ("Wf2", (F, C))):
        demo[nm] = rng.standard_normal(shp, np.float32) * .02
    for nm, w in (("bq", C), ("bk", C), ("bv", C), ("bo", C),
                  ("b1", C), ("b2", C), ("bf1", F), ("bf2", C)):
        demo[nm] = np.zeros(w, np.float32)
    demo["g1"] = np.ones(C, np.float32)
    demo["g2"] = np.ones(C, np.float32)
    print(kernel(**demo).shape)
